# revision 18
# baseline (speedup 1.0000x reference)
"""Trainium2 Bass kernel for GPUTimeMask: zero out per-batch time windows.

Semantics (matches reference):
    out = x.copy();  for m, b:  out[b, :, s[m,b] : s[m,b]+clip(w[m,b],1,150)] = 0

Strategy:
  - Shard x along the CHANNEL axis: 16 channels -> 2 per core across 8 cores.
    Every core then holds ALL 64 batch rows, so the (runtime-valued) mask
    windows live at identical local coordinates on every core -> one SPMD
    program with window offsets specialized in at build time.
  - Per core the work is a pure HBM->SBUF->HBM streaming copy of a
    [128, 60000] f32 plane (rows = batch*2 + local_channel) with ~130 tiny
    SBUF memsets (<= 2 partitions x 150 cols each) applied between load and
    store. The memsets hide entirely under the DMA stream, so the kernel
    runs at the memcpy roofline. No cross-core communication.
  - Programs are cached keyed on (starts, widths) bytes, so repeated calls
    with identical metadata skip rebuild/recompile.
"""

import sys

import numpy as np

for _p in ("/opt/trn_rl_repo",):
    if _p not in sys.path:
        sys.path.insert(0, _p)

import concourse.bass as bass
import concourse.mybir as mybir
from concourse.bass_utils import run_bass_kernel_spmd
from concourse.tile import TileContext

B, C, T = 64, 16, 60000
MAX_MASK_WIDTH = 150
N_CORES = 8
C_LOCAL = C // N_CORES          # 2 channels per core
P = B * C_LOCAL                 # 128 partitions: row = b * C_LOCAL + c_local
# Tile column ranges: a few small warmup tiles so the first stores enter
# the (single, FIFO) DMA queue early -- HBM runs ~435 GB/s only with reads
# and writes mixed (~360 GB/s read-only), so late first-stores cost time.
WARMUP = [625] * 4
STEADY = 2500
_cols = WARMUP + [STEADY] * ((T - sum(WARMUP)) // STEADY)
assert sum(_cols) == T
TILE_RANGES = []
_off = 0
for _w in _cols:
    TILE_RANGES.append((_off, _off + _w))
    _off += _w
N_BUFS = 6

_program_cache: dict[bytes, bass.Bass] = {}


def _build_program(windows: list[tuple[int, int, int]]) -> bass.Bass:
    """windows: (b, lo, hi) global column ranges to zero; identical per core."""
    nc = bass.Bass()
    x = nc.declare_dram_parameter("x", [P, T], mybir.dt.float32, isOutput=False)
    y = nc.declare_dram_parameter("y", [P, T], mybir.dt.float32, isOutput=True)
    with TileContext(nc) as tc:
        with (
            tc.tile_pool(name="const", bufs=1) as cpool,
            tc.tile_pool(name="io", bufs=N_BUFS) as pool,
        ):
            # sel[p, b] = 0.0 if p//C_LOCAL == b else 1.0.  Windows are zeroed
            # by multiplying a 32-aligned partition slab by sel[:, b] (compute
            # engines require 32-aligned partition bases, so we can't touch
            # just the 2 target partitions).  sel is built ON the vector
            # engine (no DMA) and all fixups run there too, so each
            # instruction in this codegen path carries at most one semaphore
            # wait (a hard per-instruction limit).
            sel_t = cpool.tile([P, B], mybir.dt.float32)
            tmp_t = cpool.tile([P, B], mybir.dt.float32)
            nc.gpsimd.memset(sel_t[:], 1.0)
            nc.gpsimd.memset(tmp_t[:], 1.0)
            # sel_t = 1[p >= C_LOCAL*b + C_LOCAL];  tmp_t = 1[p < C_LOCAL*b]
            nc.gpsimd.affine_select(
                sel_t[:], sel_t[:], [[-C_LOCAL, B]],
                mybir.AluOpType.is_ge, 0.0,
                base=-C_LOCAL, channel_multiplier=1,
            )
            # p < C_LOCAL*b  <=>  C_LOCAL*b - p - 1 >= 0  (is_lt unimplemented)
            nc.gpsimd.affine_select(
                tmp_t[:], tmp_t[:], [[C_LOCAL, B]],
                mybir.AluOpType.is_ge, 0.0,
                base=-1, channel_multiplier=-1,
            )
            nc.gpsimd.tensor_tensor(
                sel_t[:], sel_t[:], tmp_t[:], mybir.AluOpType.add
            )
            # DVE observes the gpsimd-built sel once here, so later fixup
            # ops don't each need a cross-engine wait slot.
            nc.vector.tensor_copy(tmp_t[:, 0:1], sel_t[:, 0:1])
            for t0, t1 in TILE_RANGES:
                tile = pool.tile([P, STEADY], mybir.dt.float32)
                tw = t1 - t0
                nc.sync.dma_start(out=tile[:, :tw], in_=x[:, t0:t1])
                for b, lo, hi in windows:
                    llo = max(lo, t0)
                    lhi = min(hi, t1)
                    if llo < lhi:
                        base = (C_LOCAL * b) // 32 * 32
                        slab = tile[base : base + 32, llo - t0 : lhi - t0]
                        nc.vector.tensor_scalar_mul(
                            slab, slab, sel_t[base : base + 32, b : b + 1]
                        )
                nc.sync.dma_start(out=y[:, t0:t1], in_=tile[:, :tw])
    return nc


def _split_multiwait(nc: bass.Bass) -> None:
    """This walrus codegen allows at most ONE sync-wait command per
    instruction.  Tile sometimes attaches several (e.g. a store waiting on
    both the fixup compute and the original load).  Hoist all but one wait
    onto standalone EventSemaphore instructions inserted just before the
    instruction on the same engine (engines execute their stream in order,
    so this preserves semantics).  We keep the compute-engine wait on DMA
    instructions (it completes last there) and hoist the DMA-queue waits.
    """
    ctr = [0]

    def mk_wait(engine, w):
        ctr[0] += 1
        ev = mybir.InstEventSemaphore(name=f"WSPLIT-{ctr[0]}")
        ev.engine = engine
        ev.sync_info = mybir.SyncInfo(on_wait=[w], on_update=[])
        return ev

    for f in nc.m.functions:
        for bb in f.blocks:
            new_insts = []
            changed = False
            for inst in bb.instructions:
                si = inst.sync_info
                ow = list(si.on_wait) if si is not None else []
                if len(ow) > 1:
                    dma_waits = [w for w in ow if "DMA" in (w.ant_name or "")]
                    other = [w for w in ow if w not in dma_waits]
                    keep = (other or dma_waits)[-1]
                    hoist = [w for w in ow if w is not keep]
                    for w in hoist:
                        new_insts.append(mk_wait(inst.engine, w))
                    inst.sync_info = mybir.SyncInfo(
                        on_wait=[keep], on_update=list(si.on_update)
                    )
                    changed = True
                new_insts.append(inst)
            if changed:
                bb.instructions = new_insts


def _get_program(starts: np.ndarray, widths: np.ndarray) -> bass.Bass:
    key = starts.tobytes() + widths.tobytes()
    prog = _program_cache.get(key)
    if prog is None:
        w = np.clip(widths, 1, MAX_MASK_WIDTH)
        # Per-b union of mask intervals (merge overlapping/adjacent)
        windows = []
        for b in range(B):
            ivs = sorted(
                (int(starts[m, b]), min(int(starts[m, b]) + int(w[m, b]), T))
                for m in range(starts.shape[0])
            )
            merged = [ivs[0]]
            for s, e in ivs[1:]:
                if s <= merged[-1][1]:
                    merged[-1] = (merged[-1][0], max(merged[-1][1], e))
                else:
                    merged.append((s, e))
            windows.extend((b, s, e) for s, e in merged if s < e)
        prog = _build_program(windows)
        _split_multiwait(prog)
        _program_cache[key] = prog
    return prog


def _run(x, starts, widths, trace=False, tmpdir=None):
    x = np.ascontiguousarray(x, dtype=np.float32)
    starts = np.asarray(starts, dtype=np.int32)
    widths = np.asarray(widths, dtype=np.int32)
    assert x.shape == (B, C, T), x.shape

    nc = _get_program(starts, widths)
    in_maps = [
        {
            "x": np.ascontiguousarray(
                x[:, k * C_LOCAL : (k + 1) * C_LOCAL, :]
            ).reshape(P, T)
        }
        for k in range(N_CORES)
    ]
    res = run_bass_kernel_spmd(
        nc, in_maps, list(range(N_CORES)), trace=trace, tmpdir=tmpdir
    )

    out = np.empty_like(x)
    for k in range(N_CORES):
        out[:, k * C_LOCAL : (k + 1) * C_LOCAL, :] = res.results[k]["y"].reshape(
            B, C_LOCAL, T
        )
    return out, res


def kernel(x, starts, widths):
    out, _ = _run(x, starts, widths, trace=False)
    return out
